# revision 9
# baseline (speedup 1.0000x reference)
"""BehaviorSequenceTransformer embedding layer on 8 TRN2 NeuronCores.

Strategy (hardcoded for B=256, S=256, W=12, HIDDEN=512):
- Data-parallel over batch: core c handles batches [32c, 32c+32) = 8192 tokens.
  Tables replicated per-core (staging untimed; gathers hit local HBM), bf16.
- MoE routing on host: tokens bucketed by branch (sku: event 1-3, url: 4,
  query: 5; PAD dropped). Within sku/url buckets, tokens are sorted by
  embedding-id chunk (32768 rows/chunk) so each chunk's rows are gathered with
  one int16 transposed dma_gather into a contiguous column range. Buckets are
  padded to 512-token tiles; pad slots scatter to a trash row.
- Event+price+bias folded into one 600-row "combo" table (event_emb,
  price_emb, constant 1.0); the matching weight chunk carries W_event,
  W_price, and the branch bias.
- cat (padded to 128 cols) + combo: per-tile transposed dma_gather (int16).
- word (12 ids/token, 100k vocab > int16): per tile, (1) chunk-compact
  non-transposed gathers (int16 chunk-local ids) into an SBUF slab,
  (2) HWDGE writeback of the slab to a DRAM bounce, (3) transposed re-gather
  from the bounce in w-major position order (int16 slab-local idx),
  (4) DVE tree-sum over the 12 w-slices (1/12 folded into the word weights).
- Matmul (token-as-M): gathered feature-major columns as stationary lhsT
  [128 feat, 128 tok], weight chunks [128 feat, 512 H] as moving rhs,
  PSUM [128 tok, 512 H] f32, ACT relu evacuation, dma_scatter_add writeback
  by original token position.
- All SWDGE DMAs round-robin over 4 queues (throughput: 1 queue ~20GB/s for
  256B descriptors, 4 queues ~75GB/s).
"""

import sys

if "/opt/trn_rl_repo" not in sys.path:
    sys.path.insert(0, "/opt/trn_rl_repo")

import numpy as np
import ml_dtypes

try:  # NTFF profile hook shim (harmless if unavailable)
    import types

    import antenv

    if "antenv.axon_hooks" not in sys.modules:
        _mod = types.ModuleType("antenv.axon_hooks")
        _mod._hook = None
        _mod.set_axon_ntff_profile_hook = lambda h: setattr(_mod, "_hook", h)
        _mod.get_axon_ntff_profile_hook = lambda: _mod._hook
        sys.modules["antenv.axon_hooks"] = _mod
        antenv.axon_hooks = _mod
        if "/root/.axon_site" not in sys.path:
            sys.path.insert(0, "/root/.axon_site")
        from trn_agent_boot.trn_boot import _ntff_profile_via_ctypes

        _mod.set_axon_ntff_profile_hook(
            _ntff_profile_via_ctypes("/opt/axon/libaxon_pjrt.so")
        )
except Exception:
    pass

import concourse.bacc as bacc
import concourse.mybir as mybir
import concourse.tile as tile
from concourse.bass_utils import run_bass_kernel_spmd

BF16 = ml_dtypes.bfloat16

N_CORES = 8
B, S, W = 256, 256, 12
H = 512
B_PER = B // N_CORES
T_PER = B_PER * S             # 8192 tokens per core
TILE = 512
TRASH = T_PER
CHUNK = 32768                 # int16-safe vocab chunk

NUM_SKU, NUM_URL, NUM_WORD, NUM_CAT = 500000, 200000, 100000, 10000
N_SKU_CH = (NUM_SKU + CHUNK - 1) // CHUNK   # 16
N_URL_CH = (NUM_URL + CHUNK - 1) // CHUNK   # 7
N_WORD_CH = (NUM_WORD + CHUNK - 1) // CHUNK  # 4

WC_SKU_SKU, WC_SKU_CAT, WC_SKU_WORD, WC_SKU_COMBO = 0, 1, 2, 3
WC_URL_URL, WC_URL_COMBO = 4, 5
WC_Q_WORD, WC_Q_COMBO = 6, 7
N_WCHUNK = 8

GQ = [0]


import os as _os

_NQROT = int(_os.environ.get("BST_NQ", "4"))


def _q():
    GQ[0] = (GQ[0] + 1) % _NQROT
    return GQ[0]


def _wrap16(ids: np.ndarray) -> np.ndarray:
    """int16 idx layout: [128, n//16], idx k at (k%16, k//16), replicated
    across the 8 groups of 16 partitions."""
    n = ids.shape[0]
    assert n % 16 == 0
    blk = ids.reshape(n // 16, 16).T.astype(np.int16)
    return np.tile(blk, (8, 1))


def _build_program(n_sku_t, n_url_t, n_q_t, caps_sku, caps_url, wslab_cap, wseg_caps):
    nc = bacc.Bacc("TRN2", target_bir_lowering=False, num_swdge_queues=4)
    f32, bf16, i16, i32 = (
        mybir.dt.float32,
        mybir.dt.bfloat16,
        mybir.dt.int16,
        mybir.dt.int32,
    )
    n_all_t = n_sku_t + n_url_t + n_q_t
    n_word_t = n_sku_t + n_q_t
    len_sku = n_sku_t * TILE
    len_url = n_url_t * TILE
    wslab_rows = wslab_cap            # slab rows per word tile (%128 == 0)
    assert wslab_cap % 128 == 0

    sku_tab = nc.dram_tensor("sku_tab", [NUM_SKU, 128], bf16, kind="ExternalInput")
    url_tab = nc.dram_tensor("url_tab", [NUM_URL, 128], bf16, kind="ExternalInput")
    word_tab = nc.dram_tensor("word_tab", [NUM_WORD, 128], bf16, kind="ExternalInput")
    cat_tab = nc.dram_tensor("cat_tab", [NUM_CAT, 128], bf16, kind="ExternalInput")
    combo_tab = nc.dram_tensor("combo_tab", [600, 128], bf16, kind="ExternalInput")
    wstack = nc.dram_tensor("wstack", [128, N_WCHUNK, H], bf16, kind="ExternalInput")
    ev_flat = nc.dram_tensor("ev_flat", [128, T_PER // 128], i32, kind="ExternalInput")

    # index tensors (per-core values)
    sku_idx = nc.dram_tensor(
        "sku_idx", [128, sum(c // 16 for c in caps_sku)], i16, kind="ExternalInput"
    )
    url_idx = nc.dram_tensor(
        "url_idx", [128, sum(c // 16 for c in caps_url)], i16, kind="ExternalInput"
    )
    cat_idx = nc.dram_tensor("cat_idx", [128, n_sku_t, 32], i16, kind="ExternalInput")
    combo_idx = nc.dram_tensor(
        "combo_idx", [128, n_all_t, 32], i16, kind="ExternalInput"
    )
    scat_idx = nc.dram_tensor("scat_idx", [128, n_all_t, 32], i16, kind="ExternalInput")
    # word first-gather: chunk-compact ids; [128, n_word_t, wslab_cap//16]
    w1_idx = nc.dram_tensor(
        "w1_idx", [128, n_word_t, wslab_cap // 16], i16, kind="ExternalInput"
    )
    # word second-gather: position-order slab indices; 12*TILE per tile
    w2_idx = nc.dram_tensor(
        "w2_idx", [128, n_word_t, 12 * TILE // 16], i16, kind="ExternalInput"
    )
    # per-(tile,chunk) first-gather segment bases in units of 1024-idx calls:
    # static schedule, so just python lists baked into the loop (host passes
    # them via closure args instead of tensors).

    bounce = nc.dram_tensor("bounce", [n_word_t, wslab_rows, 128], bf16)
    out_hbm = nc.dram_tensor("out", [T_PER + 1, H], f32, kind="ExternalOutput")
    mask_hbm = nc.dram_tensor("mask", [128, T_PER // 128], f32, kind="ExternalOutput")

    with tile.TileContext(nc) as tc:
        with (
            tc.tile_pool(name="const", bufs=1) as cpool,
            tc.tile_pool(name="gath", bufs=3) as gpool,
            tc.tile_pool(name="wslab", bufs=2) as wspool,
            tc.tile_pool(name="wgrid", bufs=2) as wgpool,
            tc.tile_pool(name="acc", bufs=8, space="PSUM") as apool,
            tc.tile_pool(name="outp", bufs=2) as opool,
        ):
            w_s = cpool.tile([128, N_WCHUNK, H], bf16)
            sku_i = cpool.tile([128, sum(c // 16 for c in caps_sku)], i16)
            url_i = cpool.tile([128, sum(c // 16 for c in caps_url)], i16)
            cat_i = cpool.tile([128, n_sku_t, 32], i16)
            combo_i = cpool.tile([128, n_all_t, 32], i16)
            scat_i = cpool.tile([128, n_all_t, 32], i16)
            w1_i = cpool.tile([128, n_word_t, wslab_cap // 16], i16)
            w2_i = cpool.tile([128, n_word_t, 12 * TILE // 16], i16)
            sku_fm = cpool.tile([128, len_sku], bf16)   # feature-major sku
            url_fm = cpool.tile([128, len_url], bf16)
            nc.sync.dma_start(w_s[:], wstack[:])
            nc.sync.dma_start(sku_i[:], sku_idx[:])
            nc.sync.dma_start(url_i[:], url_idx[:])
            nc.sync.dma_start(cat_i[:], cat_idx[:])
            nc.sync.dma_start(combo_i[:], combo_idx[:])
            nc.sync.dma_start(scat_i[:], scat_idx[:])
            nc.sync.dma_start(w1_i[:], w1_idx[:])
            nc.sync.dma_start(w2_i[:], w2_idx[:])

            # --- big feature-major gathers: sku/url, one call per chunk ----
            def chunked_fm(dst, table, idx_s, caps, nch):
                base_cols = 0
                base_idx = 0
                for c in range(nch):
                    cap = caps[c]
                    if cap == 0:
                        continue
                    lo = c * CHUNK
                    hi = min(lo + CHUNK, table.shape[0])
                    for off in range(0, cap, 768):
                        n = min(768, cap - off)
                        nc.gpsimd.dma_gather(
                            dst[:, base_cols + off : base_cols + off + n].rearrange(
                                "p (a n) -> p a n", a=1
                            ),
                            table[lo:hi, :],
                            idx_s[:, base_idx + off // 16 : base_idx + (off + n) // 16],
                            n,
                            n,
                            128,
                            transpose=True,
                            queue_num=_q(),
                        )
                    base_cols += cap
                    base_idx += cap // 16
                return base_cols

            chunked_fm(sku_fm, sku_tab, sku_i, caps_sku, N_SKU_CH)
            chunked_fm(url_fm, url_tab, url_i, caps_url, N_URL_CH)

            def word_fm_tile(ti_word):
                """word slab gather -> bounce -> position re-gather -> w-sum.
                Returns [128, TILE] bf16 feature-major word sums."""
                slab = wspool.tile([128, wslab_rows // 128, 128], bf16, tag="slab")
                base = 0
                for ci in range(N_WORD_CH):
                    cap = wseg_caps[ci]
                    lo = ci * CHUNK
                    hi = min(lo + CHUNK, NUM_WORD)
                    for off in range(0, cap, 1024):
                        n = min(1024, cap - off)
                        s0 = (base + off) // 128
                        nc.gpsimd.dma_gather(
                            slab[:, s0 : s0 + n // 128, :],
                            word_tab[lo:hi, :],
                            w1_i[
                                :,
                                ti_word,
                                (base + off) // 16 : (base + off + n) // 16,
                            ],
                            n,
                            n,
                            128,
                            transpose=False,
                            queue_num=_q(),
                        )
                    base += cap
                nc.sync.dma_start(
                    bounce[ti_word].rearrange("(s p) f -> p s f", p=128), slab[:]
                )
                grid = wgpool.tile([128, 1, 12 * TILE], bf16, tag="grid")
                for k in range(12 * TILE // 768):
                    nc.gpsimd.dma_gather(
                        grid[:, :, 768 * k : 768 * (k + 1)],
                        bounce[ti_word],
                        w2_i[:, ti_word, 48 * k : 48 * (k + 1)],
                        768,
                        768,
                        128,
                        transpose=True,
                        queue_num=_q(),
                    )
                # tree-sum the 12 w-slices into slice 0
                stride = 1
                while stride < 12:
                    for a in range(0, 12 - stride, 2 * stride):
                        nc.vector.tensor_tensor(
                            out=grid[:, 0, a * TILE : (a + 1) * TILE],
                            in0=grid[:, 0, a * TILE : (a + 1) * TILE],
                            in1=grid[:, 0, (a + stride) * TILE : (a + stride + 1) * TILE],
                            op=mybir.AluOpType.add,
                        )
                    stride *= 2
                return grid

            def run_tile(branch, ti_b, ti_all, ti_word):
                combo_g = gpool.tile([128, 1, TILE], bf16, tag="combo_g")
                nc.gpsimd.dma_gather(
                    combo_g[:], combo_tab[:], combo_i[:, ti_all, :], TILE, TILE, 128,
                    transpose=True, queue_num=_q(),
                )
                if branch == "sku":
                    cat_g = gpool.tile([128, 1, TILE], bf16, tag="cat_g")
                    nc.gpsimd.dma_gather(
                        cat_g[:], cat_tab[:], cat_i[:, ti_b, :], TILE, TILE, 128,
                        transpose=True, queue_num=_q(),
                    )
                if branch in ("sku", "query"):
                    wgrid = word_fm_tile(ti_word)

                out_s = opool.tile([128, 4, H], f32, tag="out_s")
                for s in range(4):
                    acc = apool.tile([128, H], f32, tag="acc")
                    sl = slice(TILE * ti_b + 128 * s, TILE * ti_b + 128 * (s + 1))
                    ssl = slice(128 * s, 128 * (s + 1))
                    if branch == "sku":
                        chunks = [
                            (sku_fm[:, sl], WC_SKU_SKU),
                            (cat_g[:, 0, ssl], WC_SKU_CAT),
                            (wgrid[:, 0, ssl], WC_SKU_WORD),
                            (combo_g[:, 0, ssl], WC_SKU_COMBO),
                        ]
                    elif branch == "url":
                        chunks = [
                            (url_fm[:, sl], WC_URL_URL),
                            (combo_g[:, 0, ssl], WC_URL_COMBO),
                        ]
                    else:
                        chunks = [
                            (wgrid[:, 0, ssl], WC_Q_WORD),
                            (combo_g[:, 0, ssl], WC_Q_COMBO),
                        ]
                    for ci, (lhsT, wc) in enumerate(chunks):
                        nc.tensor.matmul(
                            acc[:],
                            lhsT,
                            w_s[:, wc, :],
                            start=(ci == 0),
                            stop=(ci == len(chunks) - 1),
                        )
                    nc.scalar.activation(
                        out_s[:, s, :], acc[:], mybir.ActivationFunctionType.Relu
                    )
                nc.gpsimd.dma_scatter_add(
                    out_hbm[:], out_s[:], scat_i[:, ti_all, :], TILE, TILE, H,
                    queue_num=_q(),
                )

            ti_all = 0
            ti_word = 0
            for ti in range(n_sku_t):
                run_tile("sku", ti, ti_all, ti_word)
                ti_all += 1
                ti_word += 1
            for ti in range(n_url_t):
                run_tile("url", ti, ti_all, None)
                ti_all += 1
            for ti in range(n_q_t):
                run_tile("query", ti, ti_all, ti_word)
                ti_all += 1
                ti_word += 1

            ev_s = cpool.tile([128, T_PER // 128], i32)
            mk_s = cpool.tile([128, T_PER // 128], f32)
            nc.sync.dma_start(ev_s[:], ev_flat[:])
            nc.vector.tensor_scalar(
                out=mk_s[:],
                in0=ev_s[:],
                scalar1=0,
                scalar2=None,
                op0=mybir.AluOpType.is_equal,
            )
            nc.sync.dma_start(mask_hbm[:], mk_s[:])

    nc.compile()
    return nc


def _round_up(x, m):
    return (x + m - 1) // m * m


def kernel(
    event_table, word_table, sku_table, cat_table, price_table, url_table,
    sku_w, sku_b, url_w, url_b, query_w, query_b,
    event_type, sku_id, url_id, cat_id, price_id, word_id,
):
    # ---- tables ----------------------------------------------------------
    sku_tab = np.ascontiguousarray(np.asarray(sku_table, dtype=np.float32)).astype(BF16)
    url_tab = np.ascontiguousarray(np.asarray(url_table, dtype=np.float32)).astype(BF16)
    word_tab = np.ascontiguousarray(np.asarray(word_table, dtype=np.float32)).astype(
        BF16
    )
    cat_tab = np.zeros((NUM_CAT, 128), dtype=BF16)
    cat_tab[:, :64] = np.asarray(cat_table, dtype=np.float32).astype(BF16)

    ev_t = np.asarray(event_table, dtype=np.float32)
    pr_t = np.asarray(price_table, dtype=np.float32)
    combo = np.zeros((600, 128), dtype=np.float32)
    for e in range(6):
        combo[e * 100 : (e + 1) * 100, 0:32] = ev_t[e]
        combo[e * 100 : (e + 1) * 100, 32:64] = pr_t
    combo[:, 64] = 1.0
    combo_tab = combo.astype(BF16)

    sku_w = np.asarray(sku_w, dtype=np.float32)
    url_w = np.asarray(url_w, dtype=np.float32)
    query_w = np.asarray(query_w, dtype=np.float32)
    wstack = np.zeros((128, N_WCHUNK, H), dtype=np.float32)
    wstack[:, WC_SKU_SKU, :] = sku_w[:, 32:160].T
    wstack[:64, WC_SKU_CAT, :] = sku_w[:, 160:224].T
    wstack[:, WC_SKU_WORD, :] = sku_w[:, 256:384].T / W
    wstack[0:32, WC_SKU_COMBO, :] = sku_w[:, 0:32].T
    wstack[32:64, WC_SKU_COMBO, :] = sku_w[:, 224:256].T
    wstack[64, WC_SKU_COMBO, :] = np.asarray(sku_b, dtype=np.float32)
    wstack[:, WC_URL_URL, :] = url_w[:, 0:128].T
    wstack[0:32, WC_URL_COMBO, :] = url_w[:, 128:160].T
    wstack[64, WC_URL_COMBO, :] = np.asarray(url_b, dtype=np.float32)
    wstack[:, WC_Q_WORD, :] = query_w[:, 32:160].T / W
    wstack[0:32, WC_Q_COMBO, :] = query_w[:, 0:32].T
    wstack[64, WC_Q_COMBO, :] = np.asarray(query_b, dtype=np.float32)
    wstack = wstack.astype(BF16)

    # ---- routing ---------------------------------------------------------
    ev = np.asarray(event_type)
    sku_ids = np.asarray(sku_id)
    url_ids = np.asarray(url_id)
    cat_ids = np.asarray(cat_id)
    price_ids = np.asarray(price_id)
    word_ids = np.asarray(word_id)

    core = []
    for c in range(N_CORES):
        sl = slice(c * B_PER, (c + 1) * B_PER)
        e = ev[sl].reshape(-1)
        d = dict(
            e=e,
            sku=sku_ids[sl].reshape(-1),
            url=url_ids[sl].reshape(-1),
            cat=cat_ids[sl].reshape(-1),
            price=price_ids[sl].reshape(-1),
            word=word_ids[sl].reshape(T_PER, W),
        )
        pos_sku = np.where((e >= 1) & (e <= 3))[0]
        pos_url = np.where(e == 4)[0]
        pos_q = np.where(e == 5)[0]
        # chunk-sort sku/url buckets
        d["pos_sku"] = pos_sku[np.argsort(d["sku"][pos_sku] // CHUNK, kind="stable")]
        d["pos_url"] = pos_url[np.argsort(d["url"][pos_url] // CHUNK, kind="stable")]
        d["pos_q"] = pos_q
        core.append(d)

    # per-chunk caps (max across cores, %128) and padded bucket layouts
    def chunk_caps(key, idkey, nch):
        caps = []
        for ci in range(nch):
            mx = 0
            for d in core:
                mx = max(mx, int((d[idkey][d[key]] // CHUNK == ci).sum()))
            caps.append(_round_up(mx, 128))
        return caps

    caps_sku = chunk_caps("pos_sku", "sku", N_SKU_CH)
    caps_url = chunk_caps("pos_url", "url", N_URL_CH)
    # total padded lengths must be %TILE
    pad_sku = _round_up(sum(caps_sku), TILE) - sum(caps_sku)
    caps_sku[-1] += pad_sku
    pad_url = _round_up(sum(caps_url), TILE) - sum(caps_url)
    caps_url[-1] += pad_url
    len_sku = sum(caps_sku)
    len_url = sum(caps_url)
    n_sku_t = len_sku // TILE
    n_url_t = len_url // TILE
    max_q = max(d["pos_q"].shape[0] for d in core)
    n_q_t = max(1, _round_up(max_q, TILE) // TILE)
    n_all_t = n_sku_t + n_url_t + n_q_t
    n_word_t = n_sku_t + n_q_t

    # word slab cap: per (tile, chunk) segment caps -> one global layout.
    # For each word tile (6144 lookups), per-chunk counts vary; use caps =
    # max over (cores, tiles) per chunk, rounded so the slab is %1024.
    wseg_caps = [0] * N_WORD_CH

    def word_tile_tokens(d, ti_word, n_sku_t):
        if ti_word < n_sku_t:
            seq = padded_seq(d, "pos_sku", caps_sku, len_sku)
            return seq[ti_word * TILE : (ti_word + 1) * TILE]
        t = ti_word - n_sku_t
        seq = pad_simple(d["pos_q"], n_q_t * TILE)
        return seq[t * TILE : (t + 1) * TILE]

    def padded_seq(d, key, caps, total):
        """bucket positions laid out per-chunk with -1 padding."""
        seq = np.full(total, -1, dtype=np.int64)
        pos = d[key]
        idk = {"pos_sku": "sku", "pos_url": "url"}[key]
        chunks = d[idk][pos] // CHUNK
        base = 0
        for ci in range(len(caps)):
            sel = pos[chunks == ci]
            seq[base : base + sel.shape[0]] = sel
            base += caps[ci]
        return seq

    def pad_simple(pos, total):
        seq = np.full(total, -1, dtype=np.int64)
        seq[: pos.shape[0]] = pos
        return seq

    for d in core:
        for ti_word in range(n_word_t):
            toks = word_tile_tokens(d, ti_word, n_sku_t)
            wids = np.where(
                toks[:, None] >= 0, d["word"][np.maximum(toks, 0), :], 0
            )  # [TILE, W]; pad tokens use id 0 (harmless row)
            chunks = wids // CHUNK
            for ci in range(N_WORD_CH):
                wseg_caps[ci] = max(wseg_caps[ci], int((chunks == ci).sum()))
    wseg_caps = [_round_up(x, 128) for x in wseg_caps]
    wslab_cap = sum(wseg_caps)

    # ---- per-core device index arrays ------------------------------------
    in_maps = []
    for d in core:
        seq_sku = padded_seq(d, "pos_sku", caps_sku, len_sku)
        seq_url = padded_seq(d, "pos_url", caps_url, len_url)
        seq_q = pad_simple(d["pos_q"], n_q_t * TILE)

        def ids_for(pos_arr, table_ids, sent=0):
            return np.where(pos_arr >= 0, table_ids[np.maximum(pos_arr, 0)], sent)

        # sku/url chunk-local gather ids: -1 on pad (trailing per chunk ->
        # skipped, stale columns land in trash)
        def chunk_gather_ids(seq, table_ids, caps):
            out = []
            base = 0
            for ci, cap in enumerate(caps):
                pos_c = seq[base : base + cap]
                ids_c = np.where(
                    pos_c >= 0,
                    table_ids[np.maximum(pos_c, 0)] - ci * CHUNK,
                    0,
                )
                out.append(_wrap16(ids_c))
                base += cap
            return np.concatenate(out, axis=1)

        sku_ix = chunk_gather_ids(seq_sku, d["sku"], caps_sku)
        url_ix = chunk_gather_ids(seq_url, d["url"], caps_url)

        cat_ix = np.empty((128, n_sku_t, 32), dtype=np.int16)
        combo_ix = np.empty((128, n_all_t, 32), dtype=np.int16)
        scat_ix = np.empty((128, n_all_t, 32), dtype=np.int16)
        w1_ix = np.zeros((128, n_word_t, wslab_cap // 16), dtype=np.int16)
        w2_ix = np.zeros((128, n_word_t, 12 * TILE // 16), dtype=np.int16)

        def fill_common(pos_t, ti_all):
            cval = np.where(
                pos_t >= 0,
                d["e"][np.maximum(pos_t, 0)] * 100
                + d["price"][np.maximum(pos_t, 0)],
                0,
            )
            combo_ix[:, ti_all, :] = _wrap16(cval)
            scat_ix[:, ti_all, :] = _wrap16(np.where(pos_t >= 0, pos_t, TRASH))

        def fill_word(toks, ti_word):
            wids = np.where(toks[:, None] >= 0, d["word"][np.maximum(toks, 0), :], 0)
            # position p = w*TILE + t  (w-major grid)
            flat = wids.T.reshape(-1)  # [12*TILE], index p
            chunks = flat // CHUNK
            local = (flat - chunks * CHUNK).astype(np.int64)
            slab_slot = np.empty(12 * TILE, dtype=np.int64)
            g1 = np.zeros(wslab_cap, dtype=np.int64)
            base = 0
            for ci in range(N_WORD_CH):
                selp = np.where(chunks == ci)[0]
                slab_slot[selp] = base + np.arange(selp.shape[0])
                g1[base : base + selp.shape[0]] = local[selp]
                base += wseg_caps[ci]
            # first gather: non-transposed 1024-idx calls; within call k,
            # idx j lands at slab row 1024k + j -> [ (j%128), 8k + j//128 ].
            # But chunk boundaries break table-base per call... we instead
            # emit per-chunk ids relative to the FULL table? No: first
            # gather uses word_tab full with CHUNK-LOCAL ids? -- see below:
            # we re-add chunk bases here since the device call uses the full
            # table and ids must be global but int16-safe per chunk... they
            # are not. So the device does per-chunk calls; here we only lay
            # out the id stream per chunk segment (g1 holds local ids).
            w1_ix[:, ti_word, :] = _wrap16(g1)
            # second gather: position p reads slab row slab_slot[p]; the
            # writeback AP makes bounce row j == slab row j (identity)
            w2_ix[:, ti_word, :] = _wrap16(slab_slot)

        ti_all = 0
        ti_word = 0
        for t in range(n_sku_t):
            pos_t = seq_sku[t * TILE : (t + 1) * TILE]
            cat_ix[:, t, :] = _wrap16(ids_for(pos_t, d["cat"]))
            fill_common(pos_t, ti_all)
            fill_word(pos_t, ti_word)
            ti_all += 1
            ti_word += 1
        for t in range(n_url_t):
            pos_t = seq_url[t * TILE : (t + 1) * TILE]
            fill_common(pos_t, ti_all)
            ti_all += 1
        for t in range(n_q_t):
            pos_t = seq_q[t * TILE : (t + 1) * TILE]
            fill_common(pos_t, ti_all)
            fill_word(pos_t, ti_word)
            ti_all += 1
            ti_word += 1

        in_maps.append(
            {
                "sku_tab": sku_tab,
                "url_tab": url_tab,
                "word_tab": word_tab,
                "cat_tab": cat_tab,
                "combo_tab": combo_tab,
                "wstack": wstack,
                "ev_flat": d["e"].reshape(128, T_PER // 128).astype(np.int32),
                "sku_idx": sku_ix,
                "url_idx": url_ix,
                "cat_idx": cat_ix,
                "combo_idx": combo_ix,
                "scat_idx": scat_ix,
                "w1_idx": w1_ix,
                "w2_idx": w2_ix,
            }
        )

    nc = _build_program(n_sku_t, n_url_t, n_q_t, caps_sku, caps_url, wslab_cap, wseg_caps)
    global LAST_BUILD
    LAST_BUILD = (nc, in_maps)
    res = run_bass_kernel_spmd(nc, in_maps, core_ids=list(range(N_CORES)))

    seq = np.concatenate(
        [res.results[c]["out"][:T_PER].reshape(B_PER, S, H) for c in range(N_CORES)],
        axis=0,
    )
    mask = np.concatenate(
        [res.results[c]["mask"].reshape(B_PER, S) for c in range(N_CORES)], axis=0
    )
    return seq, mask
